# revision 3
# baseline (speedup 1.0000x reference)
"""Trainium2 Bass kernel for sorted-segment sum+mean (segment_reduce).

out[g] = concat(mean_g, sum_g) over rows of nbr_fea grouped by sorted
segment_ids; out shape [num_segments, 2*D].

Strategy
--------
Rows are sorted by segment id, so each segment is a contiguous row range.
Segments are grouped into "chunks" of S=32 consecutive segments; each chunk's
rows are packed (on host) into T row-tiles of 128 rows, laid out DMA-optimally
as [chunk][partition][tile][feat] so each chunk is one fully contiguous DMA.

On device, per 128-row tile, a one-hot matrix U[row, slot] = (rel_id == slot)
is built on the VectorEngine (is_equal against an iota constant), and the
TensorEngine computes psum[feat, slot] += x_tile.T-contract-rows @ U, i.e.
matmul(lhsT=x_tile[128,64], rhs=onehot[128,32]) accumulating over the chunk's
tiles in PSUM.  Mean = psum * (1/count) with host-baked reciprocals.  Padding
rows carry rel_id = -1 so their one-hot column is all zero.

The kernel is compiled AFTER seeing the inputs, so the (data-dependent) tile
count per chunk is a compile-time constant; one SPMD program runs on all 8
cores (each core owns 128 chunks = 4096 segments).
"""

import numpy as np

import concourse.bass as bass
import concourse.mybir as mybir
import concourse.tile as tile
from concourse import bass_utils

N_TOTAL = 4_194_304
D = 64                       # feature dim
G = 32_768                   # num segments
N_CORES = 8
S = 32                       # segment slots per chunk (matmul moving free dim)
C_TOTAL = G // S             # 1024 chunks
C = C_TOTAL // N_CORES       # 128 chunks per core
P = 128                      # rows per tile == SBUF partitions

F32 = mybir.dt.float32


def _split_syncs(nc, max_waits=1):
    """This container's walrus accepts at most one sync-wait per instruction;
    split extra waits onto preceding same-engine NoOps (engine stalls at each
    wait in turn, so semantics are identical)."""
    n_split = 0
    for f in nc.m.functions:
        for bb in f.blocks:
            new_insts = []
            for ins in bb.instructions:
                si = getattr(ins, "sync_info", None)
                waits = list(si.on_wait) if si is not None and si.on_wait else []
                if len(waits) > max_waits:
                    n_split += 1
                    extra = waits[:-max_waits]
                    for i in range(0, len(extra), max_waits):
                        nop = mybir.InstNoOp(
                            name=f"{ins.name}_wsplit{i}", ins=[], outs=[]
                        )
                        nop.engine = ins.engine
                        nop.sync_info = mybir.SyncInfo(
                            on_wait=extra[i : i + max_waits], on_update=[]
                        )
                        new_insts.append(nop)
                    si.on_wait = waits[-max_waits:]
                new_insts.append(ins)
            bb.instructions = new_insts
    return n_split


def _build_bass(T, split_syncs=True):
    """Build the SPMD program for T row-tiles per chunk."""
    nc = bass.Bass("TRN2", debug=False, num_devices=1)

    x_d = nc.dram_tensor("x", [C, P, T * D], F32, kind="ExternalInput")
    rel_d = nc.dram_tensor("rel", [P, C * T], F32, kind="ExternalInput")
    iota_d = nc.dram_tensor("iota", [P, T * S], F32, kind="ExternalInput")
    recip_d = nc.dram_tensor("recip", [D, C * S], F32, kind="ExternalInput")
    out_d = nc.dram_tensor("out", [D, C * 2 * S], F32, kind="ExternalOutput")

    with tile.TileContext(nc) as tc:
        with (
            tc.tile_pool(name="const", bufs=1) as const_pool,
            tc.tile_pool(name="xin", bufs=4) as x_pool,
            tc.tile_pool(name="oh", bufs=4) as oh_pool,
            tc.tile_pool(name="outs", bufs=1) as out_pool,
            tc.tile_pool(name="ps", bufs=4, space="PSUM") as ps_pool,
        ):
            rel_sb = const_pool.tile([P, C * T], F32)
            nc.sync.dma_start(rel_sb[:], rel_d[:])
            iota_sb = const_pool.tile([P, T * S], F32)
            nc.sync.dma_start(iota_sb[:], iota_d[:])
            recip_sb = const_pool.tile([D, C * S], F32)
            nc.sync.dma_start(recip_sb[:], recip_d[:])
            out_sb = out_pool.tile([D, C * 2 * S], F32)

            for c in range(C):
                xt = x_pool.tile([P, T * D], F32)
                nc.sync.dma_start(xt[:], x_d[c, :, :])
                oh = oh_pool.tile([P, T * S], F32)
                nc.vector.tensor_tensor(
                    oh[:],
                    rel_sb[:, c * T : (c + 1) * T].to_broadcast((P, T, S)),
                    iota_sb[:],
                    mybir.AluOpType.is_equal,
                )
                ps = ps_pool.tile([D, S], F32)
                for t in range(T):
                    nc.tensor.matmul(
                        ps[:],
                        xt[:, t * D : (t + 1) * D],
                        oh[:, t * S : (t + 1) * S],
                        start=(t == 0),
                        stop=(t == T - 1),
                    )
                base = c * 2 * S
                nc.vector.tensor_tensor(
                    out_sb[:, base : base + S],
                    ps[:],
                    recip_sb[:, c * S : (c + 1) * S],
                    mybir.AluOpType.mult,
                )
                nc.scalar.copy(out_sb[:, base + S : base + 2 * S], ps[:])

            nc.sync.dma_start(out_d[:], out_sb[:])

    if split_syncs:
        _split_syncs(nc)
    return nc


def _plan_and_pack(x, seg):
    """Host-side: chunk boundaries, tile count, packed/padded device arrays."""
    x = np.ascontiguousarray(x, dtype=np.float32)
    seg = np.asarray(seg).astype(np.int64)

    starts = np.searchsorted(seg, np.arange(0, G + 1, S)).astype(np.int64)
    n_rows = np.diff(starts)
    T = max(1, int(-(-int(n_rows.max()) // P)))  # ceil

    counts = np.bincount(seg, minlength=G).astype(np.float64)
    recip = (1.0 / np.maximum(counts, 1.0)).astype(np.float32)

    # row index for [chunk, partition, tile]: row = start_c + t*128 + p
    ridx = (
        starts[:-1][:, None, None]
        + np.arange(P, dtype=np.int64)[None, :, None]
        + (np.arange(T, dtype=np.int64) * P)[None, None, :]
    )
    valid = ridx < starts[1:][:, None, None]
    ridx_c = np.where(valid, ridx, 0)

    xbuf = x[ridx_c.reshape(-1)].reshape(C_TOTAL, P, T, D)
    xbuf[~valid] = 0.0
    xbuf = xbuf.reshape(C_TOTAL, P, T * D)

    rel_all = (seg % S).astype(np.float32)
    relbuf = np.where(valid, rel_all[ridx_c], np.float32(-1.0)).astype(np.float32)

    iota_np = np.tile(np.arange(S, dtype=np.float32), (P, T))

    in_maps = []
    for core in range(N_CORES):
        c0, c1 = core * C, (core + 1) * C
        rel_core = relbuf[c0:c1].transpose(1, 0, 2).reshape(P, C * T)
        recip_core = np.broadcast_to(
            recip[core * C * S : (core + 1) * C * S][None, :], (D, C * S)
        )
        in_maps.append(
            {
                "x": np.ascontiguousarray(xbuf[c0:c1]),
                "rel": np.ascontiguousarray(rel_core),
                "iota": iota_np,
                "recip": np.ascontiguousarray(recip_core),
            }
        )
    return T, in_maps


def _assemble(results):
    """[core]["out"] of shape [D, C*2*S] -> [G, 2*D]."""
    parts = []
    for core in range(N_CORES):
        v = results[core]["out"].reshape(D, C, 2, S)
        mean = v[:, :, 0, :].transpose(1, 2, 0).reshape(C * S, D)
        ssum = v[:, :, 1, :].transpose(1, 2, 0).reshape(C * S, D)
        parts.append(np.concatenate([mean, ssum], axis=1))
    return np.concatenate(parts, axis=0)


def _run_impl(nbr_fea, segment_ids, num_segments, trace=False, trace_kwargs=None):
    assert int(num_segments) == G, f"expected {G} segments, got {num_segments}"
    assert nbr_fea.shape == (N_TOTAL, D), nbr_fea.shape

    T, in_maps = _plan_and_pack(nbr_fea, segment_ids)
    nc = _build_bass(T)
    kw = {}
    if trace:
        kw = dict(trace=True, **(trace_kwargs or {}))
    res = bass_utils.run_bass_kernel_spmd(
        nc, in_maps, core_ids=list(range(N_CORES)), **kw
    )
    return _assemble(res.results), res


def kernel(nbr_fea, segment_ids, num_segments):
    out, _ = _run_impl(np.asarray(nbr_fea), np.asarray(segment_ids), num_segments)
    return out


# revision 6
# speedup vs baseline: 2.1238x; 2.1238x over previous
"""Trainium2 Bass kernel for sorted-segment sum+mean (segment_reduce).

out[g] = concat(mean_g, sum_g) over rows of nbr_fea grouped by sorted
segment_ids; out shape [num_segments, 2*D].

Strategy
--------
Rows are sorted by segment id, so each segment is a contiguous row range.
Segments are grouped into "chunks" of S=64 consecutive segments; each chunk's
rows are packed (on host) into T row-tiles of 128 rows, laid out DMA-optimally
as [chunk][partition][tile][feat] so each chunk is one fully contiguous DMA.

The f32 features are split on host into an exact bf16 hi/lo pair
(x = hi + lo + O(2^-18 x)), shipped side by side — same byte volume as f32 —
so the TensorEngine runs single-pass bf16 matmuls instead of 4x-slower fp32.

On device, per 128-row tile, a one-hot matrix U[row, slot] = (rel_id == slot)
is built on the VectorEngine (is_equal against an iota constant) in bf16 and
used as the matmul *stationary* operand (LDWEIGHTS of 64 cols, FWL-fast);
the moving operand is the [128 rows, hi|lo = 128] tile:
    psum[slot, 0:64]  += U.T @ hi
    psum[slot, 64:128]+= U.T @ lo
accumulated over the chunk's tiles in PSUM (fp32).  The epilogue adds the two
halves (exact sum), scales by host-baked 1/count for the mean, and stages
results in SBUF for one big output DMA.  Padding rows carry rel_id = -1 so
their one-hot row is all zero.

The kernel is compiled AFTER seeing the inputs, so the (data-dependent) tile
count per chunk is a compile-time constant; one SPMD program runs on all 8
cores (each core owns C=64 chunks = 4096 segments).
"""

import ml_dtypes
import numpy as np

import concourse.bass as bass
import concourse.mybir as mybir
import concourse.tile as tile
from concourse import bass_utils

N_TOTAL = 4_194_304
D = 64                       # feature dim
G = 32_768                   # num segments
N_CORES = 8
S = 64                       # segment slots per chunk (psum partitions)
C_TOTAL = G // S             # 512 chunks
C = C_TOTAL // N_CORES       # 64 chunks per core
P = 128                      # rows per tile == SBUF partitions

F32 = mybir.dt.float32
BF16 = mybir.dt.bfloat16
NP_BF16 = ml_dtypes.bfloat16


def _split_syncs(nc, max_waits=1):
    """This container's walrus accepts at most one sync-wait per instruction;
    split extra waits onto preceding same-engine NoOps (engine stalls at each
    wait in turn, so semantics are identical)."""
    n_split = 0
    for f in nc.m.functions:
        for bb in f.blocks:
            new_insts = []
            for ins in bb.instructions:
                si = getattr(ins, "sync_info", None)
                waits = list(si.on_wait) if si is not None and si.on_wait else []
                if len(waits) > max_waits:
                    n_split += 1
                    extra = waits[:-max_waits]
                    for i in range(0, len(extra), max_waits):
                        nop = mybir.InstNoOp(
                            name=f"{ins.name}_wsplit{i}", ins=[], outs=[]
                        )
                        nop.engine = ins.engine
                        nop.sync_info = mybir.SyncInfo(
                            on_wait=extra[i : i + max_waits], on_update=[]
                        )
                        new_insts.append(nop)
                    si.on_wait = waits[-max_waits:]
                new_insts.append(ins)
            bb.instructions = new_insts
    return n_split


def _build_bass(T, split_syncs=True):
    """Build the SPMD program for T row-tiles per chunk."""
    nc = bass.Bass("TRN2", debug=False, num_devices=1)

    x_d = nc.dram_tensor("x", [C, P, T * 2 * D], BF16, kind="ExternalInput")
    rel_d = nc.dram_tensor("rel", [P, C * T], BF16, kind="ExternalInput")
    iota_d = nc.dram_tensor("iota", [P, T * S], BF16, kind="ExternalInput")
    recip_d = nc.dram_tensor("recip", [S, C], F32, kind="ExternalInput")
    out_d = nc.dram_tensor("out", [S, C * 2 * D], F32, kind="ExternalOutput")

    with tile.TileContext(nc) as tc:
        with (
            tc.tile_pool(name="const", bufs=1) as const_pool,
            tc.tile_pool(name="xin", bufs=3) as x_pool,
            tc.tile_pool(name="oh", bufs=3) as oh_pool,
            tc.tile_pool(name="outs", bufs=1) as out_pool,
            tc.tile_pool(name="scr", bufs=3) as scr_pool,
            tc.tile_pool(name="ps", bufs=4, space="PSUM") as ps_pool,
        ):
            rel_sb = const_pool.tile([P, C * T], BF16)
            nc.sync.dma_start(rel_sb[:], rel_d[:])
            iota_sb = const_pool.tile([P, T * S], BF16)
            nc.sync.dma_start(iota_sb[:], iota_d[:])
            recip_sb = const_pool.tile([S, C], F32)
            nc.sync.dma_start(recip_sb[:], recip_d[:])
            out_sb = out_pool.tile([S, C * 2 * D], F32)

            for c in range(C):
                xt = x_pool.tile([P, T * 2 * D], BF16)
                dma_eng = nc.sync if c % 2 == 0 else nc.scalar
                dma_eng.dma_start(xt[:], x_d[c, :, :])
                oh = oh_pool.tile([P, T * S], BF16)
                nc.vector.tensor_tensor(
                    oh[:],
                    rel_sb[:, c * T : (c + 1) * T].to_broadcast((P, T, S)),
                    iota_sb[:],
                    mybir.AluOpType.is_equal,
                )
                ps = ps_pool.tile([S, 2 * D], F32)
                for t in range(T):
                    nc.tensor.matmul(
                        ps[:],
                        oh[:, t * S : (t + 1) * S],
                        xt[:, t * 2 * D : (t + 1) * 2 * D],
                        start=(t == 0),
                        stop=(t == T - 1),
                    )
                base = c * 2 * D
                # exact sum = hi-part + lo-part (walrus allows only one PSUM
                # operand per op: stage the lo half through SBUF via ACT)
                lo_sb = scr_pool.tile([S, D], F32)
                nc.scalar.copy(lo_sb[:], ps[:, D : 2 * D])
                nc.vector.tensor_tensor(
                    out_sb[:, base + D : base + 2 * D],
                    ps[:, 0:D],
                    lo_sb[:],
                    mybir.AluOpType.add,
                )
                # mean = sum * (1/count)   (per-partition scalar)
                nc.vector.tensor_scalar(
                    out_sb[:, base : base + D],
                    out_sb[:, base + D : base + 2 * D],
                    recip_sb[:, c : c + 1],
                    None,
                    mybir.AluOpType.mult,
                )

            nc.sync.dma_start(out_d[:], out_sb[:])

    if split_syncs:
        _split_syncs(nc)
    return nc


def _plan_and_pack(x, seg):
    """Host-side: chunk boundaries, tile count, packed/padded device arrays."""
    x = np.ascontiguousarray(x, dtype=np.float32)
    seg = np.asarray(seg).astype(np.int64)

    starts = np.searchsorted(seg, np.arange(0, G + 1, S)).astype(np.int64)
    n_rows = np.diff(starts)
    T = max(1, int(-(-int(n_rows.max()) // P)))  # ceil

    counts = np.bincount(seg, minlength=G).astype(np.float64)
    recip = (1.0 / np.maximum(counts, 1.0)).astype(np.float32)

    # row index for [chunk, partition, tile]: row = start_c + t*128 + p
    ridx = (
        starts[:-1][:, None, None]
        + np.arange(P, dtype=np.int64)[None, :, None]
        + (np.arange(T, dtype=np.int64) * P)[None, None, :]
    )
    valid = ridx < starts[1:][:, None, None]
    ridx_c = np.where(valid, ridx, 0)

    xg = x[ridx_c.reshape(-1)].reshape(C_TOTAL, P, T, D)
    xg[~valid] = 0.0
    hi = xg.astype(NP_BF16)
    lo = (xg - hi.astype(np.float32)).astype(NP_BF16)
    xbuf = np.empty((C_TOTAL, P, T, 2 * D), NP_BF16)
    xbuf[..., :D] = hi
    xbuf[..., D:] = lo
    del xg, hi, lo
    xbuf = xbuf.reshape(C_TOTAL, P, T * 2 * D)

    rel_all = (seg % S).astype(np.float32)
    relbuf = np.where(valid, rel_all[ridx_c], np.float32(-1.0)).astype(NP_BF16)

    iota_np = np.tile(np.arange(S, dtype=np.float32), (P, T)).astype(NP_BF16)

    in_maps = []
    for core in range(N_CORES):
        c0, c1 = core * C, (core + 1) * C
        rel_core = relbuf[c0:c1].transpose(1, 0, 2).reshape(P, C * T)
        recip_core = recip[core * C * S : (core + 1) * C * S].reshape(C, S).T
        in_maps.append(
            {
                "x": np.ascontiguousarray(xbuf[c0:c1]),
                "rel": np.ascontiguousarray(rel_core),
                "iota": iota_np,
                "recip": np.ascontiguousarray(recip_core),
            }
        )
    return T, in_maps


def _assemble(results):
    """[core]["out"] of shape [S, C*2*D] -> [G, 2*D]."""
    parts = []
    for core in range(N_CORES):
        v = results[core]["out"].reshape(S, C, 2, D)
        mean = v[:, :, 0, :].transpose(1, 0, 2).reshape(C * S, D)
        ssum = v[:, :, 1, :].transpose(1, 0, 2).reshape(C * S, D)
        parts.append(np.concatenate([mean, ssum], axis=1))
    return np.concatenate(parts, axis=0)


def _run_impl(nbr_fea, segment_ids, num_segments, trace=False, trace_kwargs=None):
    assert int(num_segments) == G, f"expected {G} segments, got {num_segments}"
    assert nbr_fea.shape == (N_TOTAL, D), nbr_fea.shape

    T, in_maps = _plan_and_pack(nbr_fea, segment_ids)
    nc = _build_bass(T)
    kw = {}
    if trace:
        kw = dict(trace=True, **(trace_kwargs or {}))
    res = bass_utils.run_bass_kernel_spmd(
        nc, in_maps, core_ids=list(range(N_CORES)), **kw
    )
    return _assemble(res.results), res


def kernel(nbr_fea, segment_ids, num_segments):
    out, _ = _run_impl(np.asarray(nbr_fea), np.asarray(segment_ids), num_segments)
    return out
